# revision 2
# baseline (speedup 1.0000x reference)
"""Trainium2 Bass kernel for a dense transformer attention layer.

Reference computation (per batch b of B=32, T=256 tokens, D=2048, 16 heads x 128):
    q = x @ wq.T ; k = x @ wk.T ; v = x @ wv.T        (torch Linear convention)
    q, k = rope(q), rope(k)
    attn = softmax(mask(q k^T / sqrt(128)))
    out  = (attn @ v) @ wo.T

Strategy: pure data parallelism over the batch dim — 4 batches per core on 8
NeuronCores, weights replicated, no collectives.  All host-side transposes are
precomputed in numpy so the device only ever does c-major matmuls:
    phase 1: Q^T, K^T (feature-major, RoPE fused into the PSUM drain) -> DRAM,
             V (token-major) -> DRAM
    phase 2: per (batch, head): S^T = K^T^T Q^T, exp, causal mask-mul,
             denominators via ones-matmul, 1/x broadcast via gpsimd,
             O^T = V^T P^T normalized -> resident SBUF (bf16)
    phase 3: y = O^T^T @ wo^T (bf16 weights) -> DRAM
All big matmuls run as float32r (1 cycle/row on the PE at N>=256).
"""

import sys

if "/opt/trn_rl_repo" not in sys.path:
    sys.path.insert(0, "/opt/trn_rl_repo")

import numpy as np

B, T, D = 32, 256, 2048
H, HD = 16, 128
NCORES = 8
BLOC = B // NCORES          # batches per core = 4
TLOC = BLOC * T             # tokens per core = 1024
ROPE_BASE = 10000.0
SCALE = 1.0 / float(np.sqrt(HD))

_CACHE = {}


def _build():
    import concourse.tile as tile
    from concourse import bacc, mybir
    from contextlib import ExitStack

    F32 = mybir.dt.float32
    F32R = mybir.dt.float32r
    BF16 = mybir.dt.bfloat16
    Exp = mybir.ActivationFunctionType.Exp

    nc = bacc.Bacc("TRN2", target_bir_lowering=False)

    xT = nc.declare_dram_parameter("xT", [D, TLOC], F32R, isOutput=False)
    wqT = nc.declare_dram_parameter("wqT", [D, D], F32R, isOutput=False)
    wkT = nc.declare_dram_parameter("wkT", [D, D], F32R, isOutput=False)
    wvT = nc.declare_dram_parameter("wvT", [D, D], F32R, isOutput=False)
    woT = nc.declare_dram_parameter("woT", [D, D], F32, isOutput=False)
    cosT = nc.declare_dram_parameter("cosT", [HD, 512], F32, isOutput=False)
    sinT = nc.declare_dram_parameter("sinT", [HD, 512], F32, isOutput=False)
    maskT = nc.declare_dram_parameter("maskT", [T, T], F32R, isOutput=False)
    onesd = nc.declare_dram_parameter("onesd", [128, 1], F32R, isOutput=False)
    y = nc.declare_dram_parameter("y", [TLOC, D], F32, isOutput=True)

    with ExitStack() as ctx:
        tc = ctx.enter_context(tile.TileContext(nc))
        # resident pools
        xpool = ctx.enter_context(tc.tile_pool(name="xpool", bufs=1))
        otpool = ctx.enter_context(tc.tile_pool(name="otpool", bufs=1))
        consts = ctx.enter_context(tc.tile_pool(name="consts", bufs=1))
        dram = ctx.enter_context(tc.tile_pool(name="dram", bufs=1, space="DRAM"))
        # streaming pools
        wst = ctx.enter_context(tc.tile_pool(name="wst", bufs=3))
        drain = ctx.enter_context(tc.tile_pool(name="drain", bufs=6))
        rtmp = ctx.enter_context(tc.tile_pool(name="rtmp", bufs=4))
        att = ctx.enter_context(tc.tile_pool(name="att", bufs=4))
        ps = ctx.enter_context(tc.tile_pool(name="ps", bufs=8, space="PSUM"))

        # ---------- resident loads ----------
        xt_sb = []
        for c in range(16):
            t_ = xpool.tile([128, TLOC], F32R, tag=f"xt{c}", name=f"xt{c}")
            nc.sync.dma_start(out=t_, in_=xT[c * 128:(c + 1) * 128, :])
            xt_sb.append(t_)

        cos_sb = consts.tile([HD, 512], F32, tag="cos", name="cos_sb")
        nc.sync.dma_start(out=cos_sb, in_=cosT[:, :])
        sin_sb = consts.tile([HD, 512], F32, tag="sin", name="sin_sb")
        nc.sync.dma_start(out=sin_sb, in_=sinT[:, :])
        mask_sb = []
        for jt in range(2):
            m_ = consts.tile([128, T], F32R, tag=f"mask{jt}", name=f"mask{jt}")
            nc.sync.dma_start(out=m_, in_=maskT[jt * 128:(jt + 1) * 128, :])
            mask_sb.append(m_)
        ones_col = consts.tile([128, 1], F32R, tag="ones", name="ones_col")
        nc.sync.dma_start(out=ones_col, in_=onesd[:, :])

        ot_sb = []
        for h in range(H):
            t_ = otpool.tile([HD, TLOC], BF16, tag=f"ot{h}", name=f"ot{h}")
            ot_sb.append(t_)

        qT_d = dram.tile([D, TLOC], F32R, tag="qT_d", name="qT_d")
        kT_d = dram.tile([D, TLOC], F32R, tag="kT_d", name="kT_d")
        v_d = dram.tile([TLOC, D], F32R, tag="v_d", name="v_d")

        # ---------- phase 1: projections ----------
        def rope_drain(psum, dst_sb):
            # psum [128(head dims), 512(tokens)] -> dst_sb (f32r) with rotary
            # applied.  Both-SBUF DVE operands must share a base partition, so
            # the rotated products live in a full [128] tile at matching rows.
            tmp = rtmp.tile([128, 512], F32, tag="rt_full", name="rt_full")
            tmp2 = rtmp.tile([128, 512], F32, tag="rt_rot", name="rt_rot")
            nc.vector.tensor_mul(tmp[:, :], psum[:, :], cos_sb[:, :])
            nc.vector.tensor_mul(tmp2[0:64, :], psum[64:128, :], sin_sb[0:64, :])
            nc.vector.tensor_mul(tmp2[64:128, :], psum[0:64, :], sin_sb[64:128, :])
            nc.vector.tensor_sub(dst_sb[0:64, :], tmp[0:64, :], tmp2[0:64, :])
            nc.vector.tensor_add(dst_sb[64:128, :], tmp[64:128, :], tmp2[64:128, :])

        for wparam, dst in ((wqT, qT_d), (wkT, kT_d)):
            for ob in range(4):          # o blocks of 512 (4 heads each)
                psums = [[ps.tile([128, 512], F32, tag="ps", name="pqk")
                          for _ in range(2)] for _ in range(4)]
                for c in range(16):
                    wt = wst.tile([128, 512], F32R, tag="wst", name="wt")
                    nc.sync.dma_start(
                        out=wt,
                        in_=wparam[c * 128:(c + 1) * 128, ob * 512:(ob + 1) * 512])
                    for oi in range(4):
                        for tb in range(2):
                            nc.tensor.matmul(
                                psums[oi][tb][:, :],
                                wt[:, oi * 128:(oi + 1) * 128],
                                xt_sb[c][:, tb * 512:(tb + 1) * 512],
                                start=(c == 0), stop=(c == 15))
                for oi in range(4):
                    for tb in range(2):
                        qsb = drain.tile([128, 512], F32R, tag="drain", name="qsb")
                        rope_drain(psums[oi][tb], qsb)
                        r0 = (ob * 4 + oi) * 128
                        nc.sync.dma_start(
                            out=dst[r0:r0 + 128, tb * 512:(tb + 1) * 512],
                            in_=qsb[:, :])

        for ob in range(4):              # V: token-major [t, o]
            psums = [ps.tile([128, 512], F32, tag="ps", name="pv") for _ in range(8)]
            for c in range(16):
                wt = wst.tile([128, 512], F32R, tag="wst", name="wtv")
                nc.sync.dma_start(
                    out=wt,
                    in_=wvT[c * 128:(c + 1) * 128, ob * 512:(ob + 1) * 512])
                for t in range(8):
                    nc.tensor.matmul(
                        psums[t][:, :],
                        xt_sb[c][:, t * 128:(t + 1) * 128],
                        wt[:, :],
                        start=(c == 0), stop=(c == 15))
            for t in range(8):
                vsb = drain.tile([128, 512], F32R, tag="drain", name="vsb")
                nc.scalar.copy(vsb[:, :], psums[t][:, :])
                nc.sync.dma_start(
                    out=v_d[t * 128:(t + 1) * 128, ob * 512:(ob + 1) * 512],
                    in_=vsb[:, :])

        # ---------- phase 2: attention ----------
        for b in range(BLOC):
            t0 = b * T
            for h in range(H):
                r0 = h * HD
                qt = att.tile([128, T], F32R, tag="qt", name="qt")
                nc.sync.dma_start(out=qt, in_=qT_d[r0:r0 + 128, t0:t0 + T])
                kt = att.tile([128, T], F32R, tag="kt", name="kt")
                nc.sync.dma_start(out=kt, in_=kT_d[r0:r0 + 128, t0:t0 + T])
                vt = [att.tile([128, HD], F32R, tag=f"vt{jt}", name="vt")
                      for jt in range(2)]
                for jt in range(2):
                    nc.sync.dma_start(
                        out=vt[jt],
                        in_=v_d[t0 + jt * 128:t0 + (jt + 1) * 128, r0:r0 + 128])

                ptm = []
                sums = ps.tile([1, T], F32, tag="ps", name="sums")
                for jt in range(2):
                    st = ps.tile([128, T], F32, tag="ps", name="st")
                    nc.tensor.matmul(st[:, :], kt[:, jt * 128:(jt + 1) * 128],
                                     qt[:, :], start=True, stop=True)
                    pt = att.tile([128, T], F32R, tag=f"pt{jt}", name="pt")
                    nc.scalar.activation(pt[:, :], st[:, :], Exp, scale=SCALE)
                    pm = att.tile([128, T], F32R, tag=f"pm{jt}", name="pm")
                    nc.vector.tensor_mul(pm[:, :], pt[:, :], mask_sb[jt][:, :])
                    ptm.append(pm)
                    nc.tensor.matmul(sums[:, :], ones_col[:, :], pm[:, :],
                                     start=(jt == 0), stop=(jt == 1))
                rec = att.tile([1, T], F32, tag="rec", name="rec")
                nc.vector.reciprocal(rec[:, :], sums[:, :])
                bc = att.tile([128, T], F32, tag="bc", name="bc")
                nc.gpsimd.partition_broadcast(bc[:, :], rec[:, :])
                ot_ps = ps.tile([128, T], F32, tag="ps", name="ot_ps")
                for jt in range(2):
                    nc.tensor.matmul(ot_ps[:, :], vt[jt][:, :], ptm[jt][:, :],
                                     start=(jt == 0), stop=(jt == 1))
                nc.vector.tensor_mul(ot_sb[h][:, t0:t0 + T], ot_ps[:, :], bc[:, :])

        # ---------- phase 3: output projection ----------
        BF = BF16
        for mb in range(4):
            psums = [ps.tile([128, 512], F32, tag="ps", name="py") for _ in range(8)]
            for e in range(16):
                wt = wst.tile([128, 512], BF, tag="wstb", name="wo_t")
                nc.gpsimd.dma_start(
                    out=wt,
                    in_=woT[e * 128:(e + 1) * 128, mb * 512:(mb + 1) * 512])
                for t in range(8):
                    nc.tensor.matmul(
                        psums[t][:, :],
                        ot_sb[e][:, t * 128:(t + 1) * 128],
                        wt[:, :],
                        start=(e == 0), stop=(e == 15))
            for t in range(8):
                ysb = drain.tile([128, 512], F32, tag="drain_y", name="ysb")
                nc.scalar.copy(ysb[:, :], psums[t][:, :])
                nc.sync.dma_start(
                    out=y[t * 128:(t + 1) * 128, mb * 512:(mb + 1) * 512],
                    in_=ysb[:, :])

    nc.compile()
    return nc


def _host_prep(x, mask, wq, wk, wv, wo):
    f32 = np.float32
    wqT = np.ascontiguousarray(np.asarray(wq, f32).T)
    wkT = np.ascontiguousarray(np.asarray(wk, f32).T)
    wvT = np.ascontiguousarray(np.asarray(wv, f32).T)
    woT = np.ascontiguousarray(np.asarray(wo, f32).T)

    inv_freq = (1.0 / (ROPE_BASE ** (np.arange(0, HD, 2, dtype=f32) / HD))).astype(f32)
    t_ = np.arange(T, dtype=f32)
    freqs = np.outer(t_, inv_freq)                    # [T, 64]
    emb = np.concatenate([freqs, freqs], axis=-1)     # [T, 128]
    cosT = np.ascontiguousarray(np.cos(emb).astype(f32).T)   # [128, T]
    sinT = np.ascontiguousarray(np.sin(emb).astype(f32).T)
    cosT = np.tile(cosT, (1, 512 // T))               # [128, 512]
    sinT = np.tile(sinT, (1, 512 // T))

    maskT = np.ascontiguousarray(np.asarray(mask).T.astype(f32))  # [T, T] {0,1}
    onesd = np.ones((128, 1), f32)

    shared = dict(wqT=wqT, wkT=wkT, wvT=wvT, woT=woT, cosT=cosT, sinT=sinT,
                  maskT=maskT, onesd=onesd)
    xf = np.asarray(x, f32)
    in_maps = []
    for i in range(NCORES):
        xs = xf[i * BLOC:(i + 1) * BLOC].reshape(TLOC, D)
        m = dict(shared)
        m["xT"] = np.ascontiguousarray(xs.T)
        in_maps.append(m)
    return in_maps


def _run(x, mask, wq, wk, wv, wo, trace=False):
    from concourse.bass_utils import run_bass_kernel_spmd

    if "nc" not in _CACHE:
        _CACHE["nc"] = _build()
    nc = _CACHE["nc"]
    in_maps = _host_prep(x, mask, wq, wk, wv, wo)
    res = run_bass_kernel_spmd(nc, in_maps, core_ids=list(range(NCORES)),
                               trace=trace)
    out = np.empty((B, T, D), np.float32)
    for i in range(NCORES):
        out[i * BLOC:(i + 1) * BLOC] = res.results[i]["y"].reshape(BLOC, T, D)
    return out, res


def kernel(x, mask, wq, wk, wv, wo):
    out, _ = _run(x, mask, wq, wk, wv, wo, trace=False)
    return out


# revision 4
# speedup vs baseline: 1.2747x; 1.2747x over previous
"""Trainium2 Bass kernel for a dense transformer attention layer.

Reference computation (per batch b of B=32, T=256 tokens, D=2048, 16 heads x 128):
    q = x @ wq.T ; k = x @ wk.T ; v = x @ wv.T        (torch Linear convention)
    q, k = rope(q), rope(k)
    attn = softmax(mask(q k^T / sqrt(128)))
    out  = (attn @ v) @ wo.T

Strategy: pure data parallelism over the batch dim — 4 batches per core on 8
NeuronCores, weights replicated, no collectives.  All host-side transposes are
precomputed in numpy so the device only ever does c-major matmuls:
    phase 1: Q^T, K^T (feature-major, RoPE fused into the PSUM drain) -> DRAM,
             V (token-major) -> DRAM
    phase 2: per (batch, head): S^T = K^T^T Q^T, exp, causal mask-mul,
             denominators via ones-matmul, 1/x broadcast via gpsimd,
             O^T = V^T P^T normalized -> resident SBUF (bf16)
    phase 3: y = O^T^T @ wo^T (bf16 weights) -> DRAM
All big matmuls run as float32r (1 cycle/row on the PE at N>=256).
"""

import sys

if "/opt/trn_rl_repo" not in sys.path:
    sys.path.insert(0, "/opt/trn_rl_repo")

import numpy as np

B, T, D = 32, 256, 2048
H, HD = 16, 128
NCORES = 8
BLOC = B // NCORES          # batches per core = 4
TLOC = BLOC * T             # tokens per core = 1024
ROPE_BASE = 10000.0
SCALE = 1.0 / float(np.sqrt(HD))

_CACHE = {}


def _build():
    import concourse.tile as tile
    from concourse import bacc, mybir
    from contextlib import ExitStack

    F32 = mybir.dt.float32
    F32R = mybir.dt.float32r
    BF16 = mybir.dt.bfloat16
    Exp = mybir.ActivationFunctionType.Exp

    nc = bacc.Bacc("TRN2", target_bir_lowering=False)

    xT = nc.declare_dram_parameter("xT", [D, TLOC], BF16, isOutput=False)
    wqT = nc.declare_dram_parameter("wqT", [D, D], BF16, isOutput=False)
    wkT = nc.declare_dram_parameter("wkT", [D, D], BF16, isOutput=False)
    wvT = nc.declare_dram_parameter("wvT", [D, D], BF16, isOutput=False)
    woT = nc.declare_dram_parameter("woT", [D, D], BF16, isOutput=False)
    cosT = nc.declare_dram_parameter("cosT", [HD, 512], F32, isOutput=False)
    sinT = nc.declare_dram_parameter("sinT", [HD, 512], F32, isOutput=False)
    maskT = nc.declare_dram_parameter("maskT", [T, T], BF16, isOutput=False)
    onesd = nc.declare_dram_parameter("onesd", [128, 1], BF16, isOutput=False)
    y = nc.declare_dram_parameter("y", [TLOC, D], F32, isOutput=True)

    with ExitStack() as ctx:
        tc = ctx.enter_context(tile.TileContext(nc))
        # resident pools
        xpool = ctx.enter_context(tc.tile_pool(name="xpool", bufs=1))
        otpool = ctx.enter_context(tc.tile_pool(name="otpool", bufs=1))
        consts = ctx.enter_context(tc.tile_pool(name="consts", bufs=1))
        dram = ctx.enter_context(tc.tile_pool(name="dram", bufs=1, space="DRAM"))
        # streaming pools
        wst = ctx.enter_context(tc.tile_pool(name="wst", bufs=8))
        drain = ctx.enter_context(tc.tile_pool(name="drain", bufs=4))
        rtmp = ctx.enter_context(tc.tile_pool(name="rtmp", bufs=3))
        att = ctx.enter_context(tc.tile_pool(name="att", bufs=3))
        ps = ctx.enter_context(tc.tile_pool(name="ps", bufs=8, space="PSUM"))

        # ---------- resident loads ----------
        xt_sb = []
        for c in range(16):
            t_ = xpool.tile([128, TLOC], BF16, tag=f"xt{c}", name=f"xt{c}")
            nc.sync.dma_start(out=t_, in_=xT[c * 128:(c + 1) * 128, :])
            xt_sb.append(t_)

        cos_sb = consts.tile([HD, 512], F32, tag="cos", name="cos_sb")
        nc.sync.dma_start(out=cos_sb, in_=cosT[:, :])
        sin_sb = consts.tile([HD, 512], F32, tag="sin", name="sin_sb")
        nc.sync.dma_start(out=sin_sb, in_=sinT[:, :])
        mask_sb = []
        for jt in range(2):
            m_ = consts.tile([128, T], BF16, tag=f"mask{jt}", name=f"mask{jt}")
            nc.sync.dma_start(out=m_, in_=maskT[jt * 128:(jt + 1) * 128, :])
            mask_sb.append(m_)
        ones_col = consts.tile([128, 1], BF16, tag="ones", name="ones_col")
        nc.sync.dma_start(out=ones_col, in_=onesd[:, :])

        ot_sb = []
        for h in range(H):
            t_ = otpool.tile([HD, TLOC], BF16, tag=f"ot{h}", name=f"ot{h}")
            ot_sb.append(t_)

        qT_d = dram.tile([D, TLOC], BF16, tag="qT_d", name="qT_d")
        kT_d = dram.tile([D, TLOC], BF16, tag="kT_d", name="kT_d")
        v_d = dram.tile([TLOC, D], BF16, tag="v_d", name="v_d")

        # ---------- phase 1: projections ----------
        def rope_drain(psum, dst_sb):
            # psum [128(head dims), 512(tokens)] -> dst_sb (f32r) with rotary
            # applied.  Both-SBUF DVE operands must share a base partition, so
            # the rotated products live in a full [128] tile at matching rows.
            tmp = rtmp.tile([128, 512], F32, tag="rt_full", name="rt_full")
            tmp2 = rtmp.tile([128, 512], F32, tag="rt_rot", name="rt_rot")
            nc.vector.tensor_mul(tmp[:, :], psum[:, :], cos_sb[:, :])
            nc.vector.tensor_mul(tmp2[0:64, :], psum[64:128, :], sin_sb[0:64, :])
            nc.vector.tensor_mul(tmp2[64:128, :], psum[0:64, :], sin_sb[64:128, :])
            nc.vector.tensor_sub(dst_sb[0:64, :], tmp[0:64, :], tmp2[0:64, :])
            nc.vector.tensor_add(dst_sb[64:128, :], tmp[64:128, :], tmp2[64:128, :])

        for wparam, dst in ((wqT, qT_d), (wkT, kT_d)):
            for ob in range(4):          # o blocks of 512 (4 heads each)
                psums = [[ps.tile([128, 512], F32, tag="ps", name="pqk")
                          for _ in range(2)] for _ in range(4)]
                for c in range(16):
                    wt = wst.tile([128, 512], BF16, tag="wst", name="wt")
                    nc.sync.dma_start(
                        out=wt,
                        in_=wparam[c * 128:(c + 1) * 128, ob * 512:(ob + 1) * 512])
                    for oi in range(4):
                        for tb in range(2):
                            nc.tensor.matmul(
                                psums[oi][tb][:, :],
                                wt[:, oi * 128:(oi + 1) * 128],
                                xt_sb[c][:, tb * 512:(tb + 1) * 512],
                                start=(c == 0), stop=(c == 15))
                for oi in range(4):
                    for tb in range(2):
                        qsb = drain.tile([128, 512], BF16, tag="drain", name="qsb")
                        rope_drain(psums[oi][tb], qsb)
                        r0 = (ob * 4 + oi) * 128
                        nc.sync.dma_start(
                            out=dst[r0:r0 + 128, tb * 512:(tb + 1) * 512],
                            in_=qsb[:, :])

        for ob in range(4):              # V: token-major [t, o]
            psums = [ps.tile([128, 512], F32, tag="ps", name="pv") for _ in range(8)]
            for c in range(16):
                wt = wst.tile([128, 512], BF16, tag="wst", name="wtv")
                nc.sync.dma_start(
                    out=wt,
                    in_=wvT[c * 128:(c + 1) * 128, ob * 512:(ob + 1) * 512])
                for t in range(8):
                    nc.tensor.matmul(
                        psums[t][:, :],
                        xt_sb[c][:, t * 128:(t + 1) * 128],
                        wt[:, :],
                        start=(c == 0), stop=(c == 15))
            for t in range(8):
                vsb = drain.tile([128, 512], BF16, tag="drain", name="vsb")
                nc.scalar.copy(vsb[:, :], psums[t][:, :])
                nc.sync.dma_start(
                    out=v_d[t * 128:(t + 1) * 128, ob * 512:(ob + 1) * 512],
                    in_=vsb[:, :])

        # ---------- phase 2: attention ----------
        for b in range(BLOC):
            t0 = b * T
            vb = [att.tile([128, D], BF16, tag=f"vb{jt}", name="vb", bufs=2)
                  for jt in range(2)]
            for jt in range(2):
                nc.sync.dma_start(
                    out=vb[jt], in_=v_d[t0 + jt * 128:t0 + (jt + 1) * 128, :])
            for h in range(H):
                r0 = h * HD
                qt = att.tile([128, T], BF16, tag="qt", name="qt")
                nc.sync.dma_start(out=qt, in_=qT_d[r0:r0 + 128, t0:t0 + T])
                kt = att.tile([128, T], BF16, tag="kt", name="kt")
                nc.sync.dma_start(out=kt, in_=kT_d[r0:r0 + 128, t0:t0 + T])
                vt = [vb[jt][:, r0:r0 + 128] for jt in range(2)]

                ptm = []
                sums = ps.tile([1, T], F32, tag="ps", name="sums")
                for jt in range(2):
                    st = ps.tile([128, T], F32, tag="ps", name="st")
                    nc.tensor.matmul(st[:, :], kt[:, jt * 128:(jt + 1) * 128],
                                     qt[:, :], start=True, stop=True)
                    pt = att.tile([128, T], BF16, tag=f"pt{jt}", name="pt")
                    nc.scalar.activation(pt[:, :], st[:, :], Exp, scale=SCALE)
                    pm = att.tile([128, T], BF16, tag=f"pm{jt}", name="pm")
                    nc.vector.tensor_mul(pm[:, :], pt[:, :], mask_sb[jt][:, :])
                    ptm.append(pm)
                    nc.tensor.matmul(sums[:, :], ones_col[:, :], pm[:, :],
                                     start=(jt == 0), stop=(jt == 1))
                rec = att.tile([1, T], F32, tag="rec", name="rec")
                nc.vector.reciprocal(rec[:, :], sums[:, :])
                bc = att.tile([128, T], F32, tag="bc", name="bc")
                nc.gpsimd.partition_broadcast(bc[:, :], rec[:, :])
                ot_ps = ps.tile([128, T], F32, tag="ps", name="ot_ps")
                for jt in range(2):
                    nc.tensor.matmul(ot_ps[:, :], vt[jt], ptm[jt][:, :],
                                     start=(jt == 0), stop=(jt == 1))
                nc.vector.tensor_mul(ot_sb[h][:, t0:t0 + T], ot_ps[:, :], bc[:, :])

        # ---------- phase 3: output projection ----------
        BF = BF16
        for mb in range(4):
            psums = [ps.tile([128, 512], F32, tag="ps", name="py") for _ in range(8)]
            for e in range(16):
                wt = wst.tile([128, 512], BF, tag="wstb", name="wo_t", bufs=6)
                nc.sync.dma_start(
                    out=wt,
                    in_=woT[e * 128:(e + 1) * 128, mb * 512:(mb + 1) * 512])
                for t in range(8):
                    nc.tensor.matmul(
                        psums[t][:, :],
                        ot_sb[e][:, t * 128:(t + 1) * 128],
                        wt[:, :],
                        start=(e == 0), stop=(e == 15))
            for t in range(8):
                ysb = drain.tile([128, 512], F32, tag="drain_y", name="ysb")
                nc.scalar.copy(ysb[:, :], psums[t][:, :])
                nc.sync.dma_start(
                    out=y[t * 128:(t + 1) * 128, mb * 512:(mb + 1) * 512],
                    in_=ysb[:, :])

    nc.compile()
    return nc


def _host_prep(x, mask, wq, wk, wv, wo):
    import ml_dtypes
    f32 = np.float32
    bf16 = ml_dtypes.bfloat16
    wqT = np.ascontiguousarray(np.asarray(wq, f32).T.astype(bf16))
    wkT = np.ascontiguousarray(np.asarray(wk, f32).T.astype(bf16))
    wvT = np.ascontiguousarray(np.asarray(wv, f32).T.astype(bf16))
    woT = np.ascontiguousarray(np.asarray(wo, f32).T.astype(bf16))

    inv_freq = (1.0 / (ROPE_BASE ** (np.arange(0, HD, 2, dtype=f32) / HD))).astype(f32)
    t_ = np.arange(T, dtype=f32)
    freqs = np.outer(t_, inv_freq)                    # [T, 64]
    emb = np.concatenate([freqs, freqs], axis=-1)     # [T, 128]
    cosT = np.ascontiguousarray(np.cos(emb).astype(f32).T)   # [128, T]
    sinT = np.ascontiguousarray(np.sin(emb).astype(f32).T)
    cosT = np.tile(cosT, (1, 512 // T))               # [128, 512]
    sinT = np.tile(sinT, (1, 512 // T))

    maskT = np.ascontiguousarray(np.asarray(mask).T.astype(bf16))  # [T, T] {0,1}
    onesd = np.ones((128, 1), bf16)

    shared = dict(wqT=wqT, wkT=wkT, wvT=wvT, woT=woT, cosT=cosT, sinT=sinT,
                  maskT=maskT, onesd=onesd)
    xf = np.asarray(x, f32)
    in_maps = []
    for i in range(NCORES):
        xs = xf[i * BLOC:(i + 1) * BLOC].reshape(TLOC, D)
        m = dict(shared)
        m["xT"] = np.ascontiguousarray(xs.T.astype(bf16))
        in_maps.append(m)
    return in_maps


def _run(x, mask, wq, wk, wv, wo, trace=False):
    from concourse.bass_utils import run_bass_kernel_spmd

    if "nc" not in _CACHE:
        _CACHE["nc"] = _build()
    nc = _CACHE["nc"]
    in_maps = _host_prep(x, mask, wq, wk, wv, wo)
    res = run_bass_kernel_spmd(nc, in_maps, core_ids=list(range(NCORES)),
                               trace=trace)
    out = np.empty((B, T, D), np.float32)
    for i in range(NCORES):
        out[i * BLOC:(i + 1) * BLOC] = res.results[i]["y"].reshape(BLOC, T, D)
    return out, res


def kernel(x, mask, wq, wk, wv, wo):
    out, _ = _run(x, mask, wq, wk, wv, wo, trace=False)
    return out
